# revision 1
# baseline (speedup 1.0000x reference)
"""Trainium2 Bass kernel for nn_Attention_33921651703853 (sparse_attention).

Data-parallel over batch: B=256 -> 32 batches on each of 8 NeuronCores.
All weights replicated; no collectives.

Device-side layout (everything transposed so no on-device transposes needed):
  - x is fed as xT tiles [d-partition, token-free], bf16, host-pretiled.
  - scores are computed transposed: sT[j, i] = sum_d k[j,d] q[i,d] * scale
    (scale folded into Wq on host). sT is evacuated PSUM->SBUF immediately
    with the mask folded in as a per-partition (j) bias, so PSUM banks free
    fast and the PE can stream ahead. The dynamic-MLP correction chains
    through TensorE with W1/W2 in natural layout; its relu+bias epilogues run
    on DVE. Softmax skips max-subtraction (logits are O(1); masked lanes get
    -1e30 -> exp -> 0).
  - AV produces outT[d, i]; the denominator row is broadcast across
    partitions via a rank-1 ones matmul, reciprocal'd on DVE
    (reciprocal_approx_fast), and fused into the PSUM->SBUF evacuation
    multiply of outT.
  - Final projection accumulates over the 8 head tiles with Wout as the
    moving operand; bout added during PSUM evacuation; output stored bf16,
    host converts back to f32.
  - Batches are processed in groups of 4 with double-buffered xT/qkT/v so
    the next group's projection matmuls overlap this group's attention phase
    (keeps TensorE dense -> HAM stays at full clock).
"""

import os
import numpy as np
import ml_dtypes

import concourse.bass as bass
import concourse.bacc as bacc
import concourse.mybir as mybir
import concourse.tile as tile
from concourse import bass_utils

BF16 = mybir.dt.bfloat16
F32 = mybir.dt.float32
AF = mybir.ActivationFunctionType
ALU = mybir.AluOpType
nbf16 = ml_dtypes.bfloat16

B, N, D, H, NK, DH = 256, 164, 1024, 8, 100, 128
NCORES = 8
BPC = B // NCORES          # 32 batches per core
GB = 4                     # batches per group
NG = BPC // GB             # 8 groups
XCOLS = GB * N             # 656 free cols per d-tile in xT sbuf
MASK_NEG = -1.0e30

_CACHE = {}
LAST_EXEC_NS = None


def _install_profile_hook():
    """Make run_bass_kernel_spmd(trace=True) work under axon in this image.

    The image's antenv package lacks axon_hooks; shim it and register the
    ctypes NTFF hook from trn_agent_boot. Also make upload_artifacts a no-op
    (zero-egress container). Returns True if tracing should work.
    """
    import sys as _sys
    import types as _types
    try:
        import antenv  # noqa: F401
        try:
            from antenv.axon_hooks import get_axon_ntff_profile_hook  # noqa: F401
        except ImportError:
            from trn_agent_boot.trn_boot import _ntff_profile_via_ctypes
            hook = _ntff_profile_via_ctypes("/opt/axon/libaxon_pjrt.so")
            mod = _types.ModuleType("antenv.axon_hooks")
            mod._hook = hook
            mod.set_axon_ntff_profile_hook = lambda h: setattr(mod, "_hook", h)
            mod.get_axon_ntff_profile_hook = lambda: mod._hook
            _sys.modules["antenv.axon_hooks"] = mod
            antenv.axon_hooks = mod
        if not getattr(bass_utils, "_upload_patched", False):
            _orig_upload = bass_utils.upload_artifacts

            def _safe_upload(tmpdir):
                try:
                    return _orig_upload(tmpdir)
                except Exception:
                    return tmpdir

            bass_utils.upload_artifacts = _safe_upload
            bass_utils._upload_patched = True
        return True
    except Exception as e:  # pragma: no cover
        print(f"profile hook install failed: {type(e).__name__}: {e}")
        return False


def _build_nc():
    nc = bacc.Bacc("TRN2", target_bir_lowering=False, debug=False)

    # ---- DRAM parameters (per-core shapes) ----
    d_xt = nc.dram_tensor("xt", [NG, 128, 8 * XCOLS], BF16, kind="ExternalInput")
    d_wqkv = nc.dram_tensor("wqkv", [128, 8 * 3 * D], BF16, kind="ExternalInput")
    d_wout = nc.dram_tensor("wout", [128, 8 * D], BF16, kind="ExternalInput")
    d_w1 = nc.dram_tensor("w1", [NK, 50], BF16, kind="ExternalInput")
    d_w2 = nc.dram_tensor("w2", [50, NK], BF16, kind="ExternalInput")
    d_b1 = nc.dram_tensor("b1c", [50, 1], F32, kind="ExternalInput")
    d_b2 = nc.dram_tensor("b2c", [NK, 1], F32, kind="ExternalInput")
    d_boutb = nc.dram_tensor("boutb", [128, D], F32, kind="ExternalInput")
    d_mbt0 = nc.dram_tensor("mbt0", [128, BPC], F32, kind="ExternalInput")
    d_mbt1 = nc.dram_tensor("mbt1", [N - 128, BPC], F32, kind="ExternalInput")
    d_xiant = nc.dram_tensor("xiant", [NG, NK, GB * NK], BF16, kind="ExternalInput")
    d_onesc = nc.dram_tensor("onesc", [128, 1], BF16, kind="ExternalInput")
    d_onesr = nc.dram_tensor("onesr", [1, 128], BF16, kind="ExternalInput")
    d_y = nc.dram_tensor("y", [BPC, N, D], BF16, kind="ExternalOutput")

    xt_ap = d_xt.ap()
    y_ap = d_y.ap()
    xiant_ap = d_xiant.ap()
    N1 = N - 128  # 36

    with tile.TileContext(nc) as tc:
        with (
            tc.tile_pool(name="const", bufs=1) as cpool,
            tc.tile_pool(name="xt", bufs=2) as xt_pool,
            tc.tile_pool(name="xian", bufs=2) as xian_pool,
            tc.tile_pool(name="qk", bufs=2) as qk_pool,
            tc.tile_pool(name="vsb", bufs=2) as v_pool,
            tc.tile_pool(name="stsb", bufs=6) as st_pool,
            tc.tile_pool(name="outT", bufs=3) as outT_pool,
            tc.tile_pool(name="ysb", bufs=3) as y_pool,
            tc.tile_pool(name="probs", bufs=6) as probs_pool,
            tc.tile_pool(name="smallsb", bufs=6) as small_pool,
            tc.tile_pool(name="pproj", bufs=2, space="PSUM") as pp,
            tc.tile_pool(name="pfast", bufs=2, space="PSUM") as pf,
            tc.tile_pool(name="pmlp", bufs=2, space="PSUM") as pm,
            tc.tile_pool(name="pout", bufs=2, space="PSUM") as po,
        ):
            # ---- load constants (ordered so group 0's projections can start
            # as soon as xt + the first Wqkv d-tiles land) ----
            wqkv_sb = cpool.tile([128, 8 * 3 * D], BF16, tag="wqkv")
            wout_sb = cpool.tile([128, 8 * D], BF16, tag="wout")
            w1_sb = cpool.tile([NK, 50], BF16, tag="w1")
            w2_sb = cpool.tile([50, NK], BF16, tag="w2")
            b1_sb = cpool.tile([50, 1], F32, tag="b1")
            b2_sb = cpool.tile([NK, 1], F32, tag="b2")
            boutb_sb = cpool.tile([128, D], F32, tag="boutb")
            mbt0_sb = cpool.tile([128, BPC], F32, tag="mbt0")
            mbt1_sb = cpool.tile([N1, BPC], F32, tag="mbt1")
            onesm_sb = cpool.tile([128, 128], BF16, tag="onesm")
            nc.vector.memset(onesm_sb[:], 1.0)

            def load_consts():
                for dt in range(8):
                    nc.sync.dma_start(
                        wqkv_sb[:, dt * 3 * D:(dt + 1) * 3 * D],
                        d_wqkv.ap()[:, dt * 3 * D:(dt + 1) * 3 * D])
                nc.sync.dma_start(w1_sb[:], d_w1.ap()[:, :])
                nc.sync.dma_start(w2_sb[:], d_w2.ap()[:, :])
                nc.sync.dma_start(b1_sb[:], d_b1.ap()[:, :])
                nc.sync.dma_start(b2_sb[:], d_b2.ap()[:, :])
                nc.sync.dma_start(mbt0_sb[:], d_mbt0.ap()[:, :])
                nc.sync.dma_start(mbt1_sb[:], d_mbt1.ap()[:, :])
                nc.sync.dma_start(wout_sb[:], d_wout.ap()[:, :])
                nc.sync.dma_start(boutb_sb[:], d_boutb.ap()[:, :])

            group_tiles = {}

            def start_group(g):
                """DMA this group's inputs, allocate its tiles, and return a
                generator of projection-chain closures (each emits one 8-MM
                accumulation chain + its evacuation)."""
                xt_sb = xt_pool.tile([128, 8 * XCOLS], BF16, tag="xt")
                nc.sync.dma_start(xt_sb[:], xt_ap[g, :, :])
                xian_sb = xian_pool.tile([NK, GB * NK], BF16, tag="xian")
                nc.sync.dma_start(xian_sb[:], xiant_ap[g, :, :])
                qkT = qk_pool.tile([128, 16 * XCOLS], BF16, tag="qkT")
                v_sb = v_pool.tile([128, GB * 2 * D], BF16, tag="v")
                group_tiles[g] = (xt_sb, qkT, v_sb, xian_sb)

                def qk_chain(ct, c0, cw):
                    pt = pp.tile([128, 512], F32, tag="proj")
                    for dt in range(8):
                        nc.tensor.matmul(
                            pt[:, :cw],
                            wqkv_sb[:, dt * 3 * D + ct * 128:dt * 3 * D + ct * 128 + 128],
                            xt_sb[:, dt * XCOLS + c0:dt * XCOLS + c0 + cw],
                            start=(dt == 0), stop=(dt == 7),
                        )
                    nc.scalar.activation(
                        qkT[:, ct * XCOLS + c0:ct * XCOLS + c0 + cw],
                        pt[:, :cw], AF.Copy,
                    )

                def v_chain(b, tt, p0, pw, ch):
                    pt = pp.tile([128, 512], F32, tag="proj")
                    for dt in range(8):
                        nc.tensor.matmul(
                            pt[:pw, :],
                            xt_sb[:, dt * XCOLS + b * N + p0:dt * XCOLS + b * N + p0 + pw],
                            wqkv_sb[:, dt * 3 * D + 2 * D + ch * 512:dt * 3 * D + 2 * D + ch * 512 + 512],
                            start=(dt == 0), stop=(dt == 7),
                        )
                    nc.vector.tensor_copy(
                        v_sb[:pw, (b * 2 + tt) * D + ch * 512:(b * 2 + tt) * D + ch * 512 + 512],
                        pt[:pw, :],
                    )

                if g < NG - 1:
                    def gen():
                        for ct in range(16):
                            for c0, cw in ((0, 512), (512, XCOLS - 512)):
                                yield lambda ct=ct, c0=c0, cw=cw: qk_chain(ct, c0, cw)
                        for b in range(GB):
                            for tt, (p0, pw) in enumerate([(0, 128), (128, N1)]):
                                for ch in range(2):
                                    yield (lambda b=b, tt=tt, p0=p0, pw=pw, ch=ch:
                                           v_chain(b, tt, p0, pw, ch))

                    return gen(), None

                # Last group: chunk per batch so later batches' chains can be
                # zipped into this group's own attention (nothing else covers
                # its tail). lists[b] must complete before attention(b).
                lists = []
                for b in range(GB):
                    L = []
                    for ct in range(16):
                        L.append(lambda ct=ct, b=b: qk_chain(ct, b * N, N))
                    for tt, (p0, pw) in enumerate([(0, 128), (128, N1)]):
                        for ch in range(2):
                            L.append(lambda b=b, tt=tt, p0=p0, pw=pw, ch=ch:
                                     v_chain(b, tt, p0, pw, ch))
                    lists.append(L)
                return iter(lists[0] + lists[1]), lists[2] + lists[3]

            # prologue: group 0's input DMAs go out first, then the weights,
            # then group 0's projection chains
            g0_chains, _ = start_group(0)
            load_consts()
            for chain in g0_chains:
                chain()
            group_own = {}

            def final_chain(outT, gb, it):
                i0, iw = (0, 128) if it == 0 else (128, N1)
                y_sb = y_pool.tile([128, D], BF16, tag="y")
                for ch in range(2):
                    yp = pp.tile([128, 512], F32, tag="proj")
                    for h2 in range(H):
                        nc.tensor.matmul(
                            yp[:iw, :],
                            outT[:, h2 * N + i0:h2 * N + i0 + iw],
                            wout_sb[:, h2 * D + ch * 512:h2 * D + ch * 512 + 512],
                            start=(h2 == 0), stop=(h2 == 7),
                        )
                    nc.vector.tensor_add(
                        y_sb[:iw, ch * 512:ch * 512 + 512],
                        yp[:iw, :],
                        boutb_sb[:iw, ch * 512:ch * 512 + 512],
                    )
                nc.sync.dma_start(y_ap[gb, i0:i0 + iw, :], y_sb[:iw, :])

            pending_final = None

            for g in range(NG):
                xt_sb, qkT, v_sb, xian_sb = group_tiles.pop(g)
                if g + 1 < NG:
                    nxt, tail = start_group(g + 1)
                    n_nxt = 48 if g + 1 < NG - 1 else 40
                    if tail is not None:
                        group_own[g + 1] = tail
                else:
                    nxt, n_nxt = iter(()), 48
                own = group_own.pop(g, None)
                own_emitted = 0
                emitted = 0
                hsteps = 0

                # ---- attention per (batch, head), zipped with next group's
                # projection chains so the PE stream stays dense ----
                for b in range(GB):
                    gb = g * GB + b
                    outT = outT_pool.tile([128, 8 * N], BF16, tag="outT")
                    for h in range(H):
                        qof = h * XCOLS + b * N
                        kof = (8 + h) * XCOLS + b * N
                        # scoresT = k @ q^T (scale pre-folded into Wq);
                        # evacuate to SBUF immediately with mask bias folded in.
                        sp0 = pf.tile([128, N], F32, tag="ps")
                        nc.tensor.matmul(sp0[:], qkT[:, kof:kof + 128], qkT[:, qof:qof + N])
                        sT0 = st_pool.tile([128, N], F32, tag="sT")
                        nc.scalar.activation(sT0[:], sp0[:], AF.Identity,
                                             bias=mbt0_sb[:, gb:gb + 1])
                        sp1 = pf.tile([128, N], F32, tag="ps")
                        nc.tensor.matmul(sp1[:N1, :], qkT[:, kof + 128:kof + N], qkT[:, qof:qof + N])
                        sT1 = st_pool.tile([N1, N], F32, tag="sT1")
                        nc.scalar.activation(sT1[:], sp1[:N1, :], AF.Identity,
                                             bias=mbt1_sb[:, gb:gb + 1])
                        # dynamic MLP on the keypoint block (all transposed)
                        raqT = small_pool.tile([NK, NK], BF16, tag="raqT")
                        nc.vector.tensor_scalar(raqT[:], sT0[:NK, :NK], 0.0, None, ALU.max)
                        m1T = pm.tile([128, N], F32, tag="pm")
                        nc.tensor.matmul(m1T[:50, :NK], w1_sb[:, :], raqT[:])
                        h1T = small_pool.tile([50, NK], BF16, tag="h1T")
                        nc.vector.tensor_scalar(h1T[:], m1T[:50, :NK], b1_sb[:], 0.0,
                                                ALU.add, ALU.max)
                        m2T = pm.tile([128, N], F32, tag="pm")
                        nc.tensor.matmul(m2T[:NK, :NK], w2_sb[:, :], h1T[:])
                        lvT = small_pool.tile([NK, NK], BF16, tag="lvT")
                        nc.vector.tensor_scalar(lvT[:], m2T[:NK, :NK], b2_sb[:], 0.0,
                                                ALU.add, ALU.max)
                        tmp = small_pool.tile([NK, NK], F32, tag="tmp")
                        nc.vector.tensor_mul(tmp[:], xian_sb[:, b * NK:(b + 1) * NK], lvT[:])
                        nc.vector.tensor_add(sT0[:NK, :NK], sT0[:NK, :NK], tmp[:])
                        # exp (no max subtraction; mask already folded in)
                        probsT0 = probs_pool.tile([128, N], BF16, tag="probs")
                        nc.scalar.activation(probsT0[:], sT0[:], AF.Exp)
                        probsT1 = probs_pool.tile([128, N], BF16, tag="probs")
                        nc.scalar.activation(probsT1[:N1, :], sT1[:], AF.Exp)
                        # denominator: ones-matrix matmul computes the column
                        # sums already broadcast across all 128 partitions
                        dbc = pm.tile([128, N], F32, tag="pm")
                        nc.tensor.matmul(dbc[:], onesm_sb[:, :], probsT0[:],
                                         start=True, stop=False)
                        nc.tensor.matmul(dbc[:], onesm_sb[:N1, :], probsT1[:N1, :],
                                         start=False, stop=True)
                        rbc_sb = small_pool.tile([128, N], F32, tag="rbcsb")
                        nc.vector.reciprocal_approx_fast(rbc_sb[:], dbc[:])
                        # AV (accumulate the two j-tiles), normalize on evacuation
                        oT = po.tile([128, N], F32, tag="oT")
                        nc.tensor.matmul(oT[:], v_sb[:, (b * 2) * D + h * DH:(b * 2) * D + h * DH + DH],
                                         probsT0[:], start=True, stop=False)
                        nc.tensor.matmul(oT[:], v_sb[:N1, (b * 2 + 1) * D + h * DH:(b * 2 + 1) * D + h * DH + DH],
                                         probsT1[:N1, :], start=False, stop=True)
                        nc.vector.tensor_mul(outT[:, h * N:h * N + N], oT[:], rbc_sb[:])
                        # last group has no next-group chains to zip; spread
                        # its deferred finals across the heads instead
                        if g == NG - 1 and h in (2, 6) and pending_final is not None:
                            final_chain(*pending_final, h // 4)
                            if h == 6:
                                pending_final = None
                        # zip in this group's own deferred chains (last group)
                        hsteps += 1
                        if own is not None:
                            tgt = len(own) if hsteps > 24 else (hsteps * len(own)) // 24
                            while own_emitted < min(tgt, len(own)):
                                own[own_emitted]()
                                own_emitted += 1
                        # zip in next group's projection chains
                        while emitted < (hsteps * n_nxt) // (GB * H):
                            chain = next(nxt, None)
                            if chain is None:
                                break
                            chain()
                            emitted += 1

                    # final projection is software-pipelined one batch back:
                    # the previous batch's final lands here, right in this
                    # batch's last-head serial-chain bubble
                    if pending_final is not None:
                        final_chain(*pending_final, 0)
                        final_chain(*pending_final, 1)
                    pending_final = (outT, gb)

                # drain any leftover projection chains for the next group
                for chain in nxt:
                    chain()

            if pending_final is not None:
                final_chain(*pending_final, 0)
                final_chain(*pending_final, 1)

    nc.compile()
    return nc


def _prep_core_inputs(xc, maskc, xianc, wqkv_h, wout_h, w1_h, w2_h, b1_h, b2_h, boutb_h,
                      onesc_h, onesr_h):
    # xT tiles: [BPC,N,D] -> (g, p, dt, b, n) -> [NG, 128, 8*GB*N]
    xt = xc.transpose(0, 2, 1).reshape(NG, GB, 8, 128, N)
    xt = np.ascontiguousarray(xt.transpose(0, 3, 2, 1, 4)).reshape(NG, 128, 8 * GB * N)
    xt = xt.astype(nbf16)
    # mask bias transposed: [164, BPC]
    mb = np.where(maskc, np.float32(MASK_NEG), np.float32(0.0)).astype(np.float32)
    mbt = np.ascontiguousarray(mb.T)
    # xianT: [BPC,100,100] -> xianT[b][j,i] = xian[b][i,j] -> (g, j, b, i)
    xiant = xianc.transpose(0, 2, 1).reshape(NG, GB, NK, NK)
    xiant = np.ascontiguousarray(xiant.transpose(0, 2, 1, 3)).reshape(NG, NK, GB * NK)
    xiant = xiant.astype(nbf16)
    return {
        "xt": xt,
        "wqkv": wqkv_h,
        "wout": wout_h,
        "w1": w1_h,
        "w2": w2_h,
        "b1c": b1_h,
        "b2c": b2_h,
        "boutb": boutb_h,
        "mbt0": np.ascontiguousarray(mbt[:128]),
        "mbt1": np.ascontiguousarray(mbt[128:]),
        "xiant": xiant,
        "onesc": onesc_h,
        "onesr": onesr_h,
    }


def kernel(x, mask, xian, Wqkv, W1, b1, W2, b2, Wout, bout):
    global LAST_EXEC_NS
    x = np.asarray(x, dtype=np.float32)
    mask = np.asarray(mask)
    xian = np.asarray(xian, dtype=np.float32)
    Wqkv = np.asarray(Wqkv, dtype=np.float32)
    W1 = np.asarray(W1, dtype=np.float32)
    b1 = np.asarray(b1, dtype=np.float32)
    W2 = np.asarray(W2, dtype=np.float32)
    b2 = np.asarray(b2, dtype=np.float32)
    Wout = np.asarray(Wout, dtype=np.float32)
    bout = np.asarray(bout, dtype=np.float32)

    if "nc" not in _CACHE:
        _CACHE["nc"] = _build_nc()
    nc = _CACHE["nc"]

    # ---- shared weight prep (scale folded into Wq) ----
    scale = np.float32(D ** -0.5)
    wqkv_s = Wqkv.copy()
    wqkv_s[:, :D] *= scale
    wqkv_h = np.ascontiguousarray(
        wqkv_s.reshape(8, 128, 3 * D).transpose(1, 0, 2)).reshape(128, 8 * 3 * D).astype(nbf16)
    wout_h = np.ascontiguousarray(
        Wout.reshape(8, 128, D).transpose(1, 0, 2)).reshape(128, 8 * D).astype(nbf16)
    w1_h = W1.astype(nbf16)
    w2_h = W2.astype(nbf16)
    b1_h = np.ascontiguousarray(b1.reshape(50, 1))
    b2_h = np.ascontiguousarray(b2.reshape(NK, 1))
    boutb_h = np.ascontiguousarray(np.broadcast_to(bout, (128, D))).astype(np.float32)
    onesc_h = np.ones((128, 1), dtype=nbf16)
    onesr_h = np.ones((1, 128), dtype=nbf16)

    in_maps = []
    for c in range(NCORES):
        sl = slice(c * BPC, (c + 1) * BPC)
        in_maps.append(_prep_core_inputs(
            x[sl], mask[sl], xian[sl], wqkv_h, wout_h, w1_h, w2_h,
            b1_h, b2_h, boutb_h, onesc_h, onesr_h))

    trace = bool(int(os.environ.get("KERNEL_TRACE", "0")))
    if trace:
        trace = _install_profile_hook()
    res = bass_utils.run_bass_kernel_spmd(
        nc, in_maps, core_ids=list(range(NCORES)), trace=trace)
    LAST_EXEC_NS = res.exec_time_ns

    out = np.empty((B, N, D), dtype=np.float32)
    for c in range(NCORES):
        out[c * BPC:(c + 1) * BPC] = res.results[c]["y"].astype(np.float32)
    return out



# revision 9
# speedup vs baseline: 1.4115x; 1.4115x over previous
"""Trainium2 Bass kernel for nn_Attention_33921651703853 (sparse_attention).

Data-parallel over batch: B=256 -> 32 batches on each of 8 NeuronCores.
All weights replicated; no collectives.

Design (v2 — software-pipelined, head-batched attention):
  - Everything transposed on device (no on-device transposes):
    xT tiles [d-part, token-free] bf16, host-pretiled; scores computed as
    sT[j, i]; AV gives outT[d, i]; final projection consumes outT directly.
  - Attention is processed per batch with all 8 heads batched:
    scores for head-groups (3,3,2) land in shared PSUM tiles and are
    evacuated in one activation per group (mask folded as per-partition
    bias). The keypoint-block MLP runs once per batch over [100, 8*100]
    (W1/W2 stationary reused), its epilogues are single DVE ops, xian is
    broadcast across heads with a stride-0 AP. exp runs as one activation
    over [128, 8*164]; the j-tail (rows 128:164) goes psum->exp directly
    (bias folded into Exp), skipping an SBUF staging tile.
  - Softmax denominators: ones-matmul broadcast trick, batched over all 8
    heads (3 chunks of <=512 cols), tail rows accumulated into the same
    PSUM group. Reciprocal on DVE; AV results are normalized during the
    PSUM->SBUF evacuation multiply (per 3-head group).
  - Batches are software-pipelined: step i emits phase A(i) (scores+MLP+exp)
    interleaved with phase B(i-1) (denom+AV) of the previous batch, so no
    engine ever waits on a freshly produced dependency.
  - Final projection is packed across batch PAIRS: outT pair tiles
    [128, 8h*328] give i-tiles (128,128,72) instead of (128,36)x2 -> 25%
    fewer streamed PE columns. y rows are contiguous global tokens.
  - QKV weights are stored ct-major (wqk) so the first projection chain only
    waits on a 256KB DMA, not the full 6MB. Projection/final chains of the
    next group are zipped into the current group's attention stream with
    deadline pacing to keep TensorE gapless (p-state stays at 2.4GHz).
"""

import os
from collections import deque
import numpy as np
import ml_dtypes

import concourse.bass as bass
import concourse.bacc as bacc
import concourse.mybir as mybir
import concourse.tile as tile
from concourse import bass_utils

BF16 = mybir.dt.bfloat16
F32 = mybir.dt.float32
AF = mybir.ActivationFunctionType
ALU = mybir.AluOpType
nbf16 = ml_dtypes.bfloat16

B, N, D, H, NK, DH = 256, 164, 1024, 8, 100, 128
NCORES = 8
BPC = B // NCORES          # 32 batches per core
GB = 4                     # batches per group
NG = BPC // GB             # 8 groups
XCOLS = GB * N             # 656 free cols per d-tile in xT sbuf
PW = 2 * N                 # 328 pair width (final projection packing)
N1 = N - 128               # 36 tail rows
MASK_NEG = -1.0e30
HGRP = [(0, 3), (3, 3), (6, 2)]   # head groups for score/AV psum batching

_CACHE = {}
LAST_EXEC_NS = None


def _install_profile_hook():
    """Make run_bass_kernel_spmd(trace=True) work under axon in this image."""
    import sys as _sys
    import types as _types
    try:
        import antenv  # noqa: F401
        try:
            from antenv.axon_hooks import get_axon_ntff_profile_hook  # noqa: F401
        except ImportError:
            from trn_agent_boot.trn_boot import _ntff_profile_via_ctypes
            hook = _ntff_profile_via_ctypes("/opt/axon/libaxon_pjrt.so")
            mod = _types.ModuleType("antenv.axon_hooks")
            mod._hook = hook
            mod.set_axon_ntff_profile_hook = lambda h: setattr(mod, "_hook", h)
            mod.get_axon_ntff_profile_hook = lambda: mod._hook
            _sys.modules["antenv.axon_hooks"] = mod
            antenv.axon_hooks = mod
        if not getattr(bass_utils, "_upload_patched", False):
            _orig_upload = bass_utils.upload_artifacts

            def _safe_upload(tmpdir):
                try:
                    return _orig_upload(tmpdir)
                except Exception:
                    return tmpdir

            bass_utils.upload_artifacts = _safe_upload
            bass_utils._upload_patched = True
        return True
    except Exception as e:  # pragma: no cover
        print(f"profile hook install failed: {type(e).__name__}: {e}")
        return False


def _build_nc():
    nc = bacc.Bacc("TRN2", target_bir_lowering=False, debug=False)

    # ---- DRAM parameters (per-core shapes) ----
    d_xt = nc.dram_tensor("xt", [NG, 128, 8 * XCOLS], BF16, kind="ExternalInput")
    d_wqk = nc.dram_tensor("wqk", [128, 16 * 1024], BF16, kind="ExternalInput")
    d_wv = nc.dram_tensor("wv", [128, 8 * 1024], BF16, kind="ExternalInput")
    d_wout = nc.dram_tensor("wout", [128, 8 * D], BF16, kind="ExternalInput")
    d_w1 = nc.dram_tensor("w1", [NK, 50], BF16, kind="ExternalInput")
    d_w2 = nc.dram_tensor("w2", [50, NK], BF16, kind="ExternalInput")
    d_b1 = nc.dram_tensor("b1c", [50, 1], F32, kind="ExternalInput")
    d_b2 = nc.dram_tensor("b2c", [NK, 1], F32, kind="ExternalInput")
    d_boutb = nc.dram_tensor("boutb", [128, D], F32, kind="ExternalInput")
    d_mbt0 = nc.dram_tensor("mbt0", [128, BPC], F32, kind="ExternalInput")
    d_mbt1 = nc.dram_tensor("mbt1", [N1, BPC], F32, kind="ExternalInput")
    d_xiant = nc.dram_tensor("xiant", [NG, NK, GB * NK], BF16, kind="ExternalInput")
    d_y = nc.dram_tensor("y", [BPC * N, D], BF16, kind="ExternalOutput")

    xt_ap = d_xt.ap()
    y_ap = d_y.ap()
    xiant_ap = d_xiant.ap()

    with tile.TileContext(nc) as tc:
        with (
            tc.tile_pool(name="const", bufs=1) as cpool,
            tc.tile_pool(name="xt", bufs=2) as xt_pool,
            tc.tile_pool(name="xian", bufs=2) as xian_pool,
            tc.tile_pool(name="qk", bufs=2) as qk_pool,
            tc.tile_pool(name="vsb", bufs=2) as v_pool,
            tc.tile_pool(name="st0", bufs=1) as st0_pool,
            tc.tile_pool(name="pr0", bufs=2) as pr0_pool,
            tc.tile_pool(name="pr1", bufs=2) as pr1_pool,
            tc.tile_pool(name="mlp", bufs=2) as mlp_pool,
            tc.tile_pool(name="rbcp", bufs=1) as rbc_pool,
            tc.tile_pool(name="outT", bufs=2) as outT_pool,
            tc.tile_pool(name="ysb", bufs=2) as y_pool,
            tc.tile_pool(name="pproj", bufs=2, space="PSUM") as pp,
            tc.tile_pool(name="psc", bufs=3, space="PSUM") as psc,
            tc.tile_pool(name="pmx", bufs=3, space="PSUM") as pm,
        ):
            # ---- constant tiles ----
            wqk_sb = cpool.tile([128, 16 * 1024], BF16, tag="wqk")
            wv_sb = cpool.tile([128, 8 * 1024], BF16, tag="wv")
            wout_sb = cpool.tile([128, 8 * D], BF16, tag="wout")
            w1_sb = cpool.tile([NK, 50], BF16, tag="w1")
            w2_sb = cpool.tile([50, NK], BF16, tag="w2")
            b1_sb = cpool.tile([50, 1], F32, tag="b1")
            b2_sb = cpool.tile([NK, 1], F32, tag="b2")
            boutb_sb = cpool.tile([128, D], F32, tag="boutb")
            mbt0_sb = cpool.tile([128, BPC], F32, tag="mbt0")
            mbt1_sb = cpool.tile([N1, BPC], F32, tag="mbt1")
            onesm_sb = cpool.tile([128, 128], BF16, tag="onesm")
            nc.vector.memset(onesm_sb[:], 1.0)

            def load_consts_early():
                # small tiles needed by the first attention batch
                nc.sync.dma_start(w1_sb[:], d_w1.ap()[:, :])
                nc.sync.dma_start(w2_sb[:], d_w2.ap()[:, :])
                nc.sync.dma_start(b1_sb[:], d_b1.ap()[:, :])
                nc.sync.dma_start(b2_sb[:], d_b2.ap()[:, :])
                nc.sync.dma_start(mbt0_sb[:], d_mbt0.ap()[:, :])
                nc.sync.dma_start(mbt1_sb[:], d_mbt1.ap()[:, :])

            def load_consts_late():
                nc.sync.dma_start(wv_sb[:, :4096], d_wv.ap()[:, :4096])
                nc.sync.dma_start(wv_sb[:, 4096:], d_wv.ap()[:, 4096:])
                nc.sync.dma_start(wout_sb[:, :4096], d_wout.ap()[:, :4096])
                nc.sync.dma_start(wout_sb[:, 4096:], d_wout.ap()[:, 4096:])
                nc.sync.dma_start(boutb_sb[:], d_boutb.ap()[:, :])

            # ---- projection chains ----
            group_tiles = {}

            def start_group(g):
                """Issue group g's input DMAs, allocate tiles, return the list
                of projection-chain closures (qk then v)."""
                xt_sb = xt_pool.tile([128, 8 * XCOLS], BF16, tag="xt")
                nc.sync.dma_start(xt_sb[:], xt_ap[g, :, :])
                xian_sb = xian_pool.tile([NK, GB * NK], BF16, tag="xian")
                nc.sync.dma_start(xian_sb[:], xiant_ap[g, :, :])
                qkT = qk_pool.tile([128, 16 * XCOLS], BF16, tag="qkT")
                v_sb = v_pool.tile([128, GB * 2 * D], BF16, tag="v")
                group_tiles[g] = (xt_sb, qkT, v_sb, xian_sb)

                def qk_chain(ct, c0, cw):
                    pt = pp.tile([128, 512], F32, tag="proj")
                    for dt in range(8):
                        nc.tensor.matmul(
                            pt[:, :cw],
                            wqk_sb[:, ct * 1024 + dt * 128:ct * 1024 + dt * 128 + 128],
                            xt_sb[:, dt * XCOLS + c0:dt * XCOLS + c0 + cw],
                            start=(dt == 0), stop=(dt == 7),
                        )
                    nc.scalar.activation(
                        qkT[:, ct * XCOLS + c0:ct * XCOLS + c0 + cw],
                        pt[:, :cw], AF.Copy,
                    )

                def v_chain(b, tt, ch):
                    p0, pw = (0, 128) if tt == 0 else (128, N1)
                    pt = pp.tile([128, 512], F32, tag="proj")
                    for dt in range(8):
                        nc.tensor.matmul(
                            pt[:pw, :],
                            xt_sb[:, dt * XCOLS + b * N + p0:dt * XCOLS + b * N + p0 + pw],
                            wv_sb[:, dt * 1024 + ch * 512:dt * 1024 + ch * 512 + 512],
                            start=(dt == 0), stop=(dt == 7),
                        )
                    nc.vector.tensor_copy(
                        v_sb[:pw, (b * 2 + tt) * D + ch * 512:(b * 2 + tt) * D + ch * 512 + 512],
                        pt[:pw, :],
                    )

                if g < NG - 1:
                    chains = []
                    for ct in range(16):
                        for c0, cw in ((0, 512), (512, XCOLS - 512)):
                            chains.append(lambda ct=ct, c0=c0, cw=cw: qk_chain(ct, c0, cw))
                    for b in range(GB):
                        for tt in range(2):
                            for ch in range(2):
                                chains.append(lambda b=b, tt=tt, ch=ch: v_chain(b, tt, ch))
                    return chains, None
                # last group: split per pair; pair chains must finish before
                # that pair's attention
                pairs = []
                for p in range(2):
                    L = []
                    for ct in range(16):
                        L.append(lambda ct=ct, p=p: qk_chain(ct, p * PW, PW))
                    for b in (2 * p, 2 * p + 1):
                        for tt in range(2):
                            for ch in range(2):
                                L.append(lambda b=b, tt=tt, ch=ch: v_chain(b, tt, ch))
                    pairs.append(L)
                return pairs[0], pairs[1]

            # ---- final projection (packed per batch pair) ----
            def final_chain(outT_p, p, it):
                i0 = it * 128
                iw = 128 if it < 2 else PW - 256
                for ch in range(2):
                    y_sb = y_pool.tile([128, 512], BF16, tag="y")
                    yp = pp.tile([128, 512], F32, tag="proj")
                    for h2 in range(H):
                        nc.tensor.matmul(
                            yp[:iw, :],
                            outT_p[:, h2 * PW + i0:h2 * PW + i0 + iw],
                            wout_sb[:, h2 * D + ch * 512:h2 * D + ch * 512 + 512],
                            start=(h2 == 0), stop=(h2 == 7),
                        )
                    nc.vector.tensor_add(
                        y_sb[:iw, :],
                        yp[:iw, :],
                        boutb_sb[:iw, ch * 512:ch * 512 + 512],
                    )
                    nc.sync.dma_start(
                        y_ap[p * PW + i0:p * PW + i0 + iw, ch * 512:ch * 512 + 512],
                        y_sb[:iw, :])

            # ---- work queues & pacing ----
            work = deque()
            finals = deque()

            def pump(n):
                # chains have group-window deadlines -> drain them first
                while n > 0 and (work or finals):
                    q = work if work else finals
                    q.popleft()()
                    n -= 1

            def pump_finals(n):
                while n > 0 and finals:
                    finals.popleft()()
                    n -= 1

            # ---- attention phase A: scores + MLP + exp for batch gi ----
            state = {}

            def emit_A(gi, pumps):
                g, b = divmod(gi, GB)
                xt_sb, qkT, v_sb, xian_sb = group_tiles[g]
                half = b & 1
                if half == 0:
                    outT_p = outT_pool.tile([128, H * PW], BF16, tag="outT")
                else:
                    outT_p = state[gi - 1][2]
                sT0 = st0_pool.tile([128, H * N], F32, tag="sT0")
                probs0 = pr0_pool.tile([128, H * N], BF16, tag="p0")
                probs1 = pr1_pool.tile([N1, H * N], BF16, tag="p1")
                # scores per head group
                for h0, gw in HGRP:
                    sp0 = psc.tile([128, 492], F32, tag="sc")
                    sp1 = pm.tile([128, 512], F32, tag="pm")
                    for k in range(gw):
                        h = h0 + k
                        qof = h * XCOLS + b * N
                        kof = (8 + h) * XCOLS + b * N
                        nc.tensor.matmul(sp0[:, k * N:k * N + N],
                                         qkT[:, kof:kof + 128],
                                         qkT[:, qof:qof + N])
                        nc.tensor.matmul(sp1[:N1, k * N:k * N + N],
                                         qkT[:, kof + 128:kof + N],
                                         qkT[:, qof:qof + N])
                    nc.scalar.activation(sT0[:, h0 * N:(h0 + gw) * N],
                                         sp0[:, :gw * N], AF.Identity,
                                         bias=mbt0_sb[:, gi:gi + 1])
                    # tail rows: fold mask into exp, psum -> probs directly
                    nc.scalar.activation(probs1[:, h0 * N:(h0 + gw) * N],
                                         sp1[:N1, :gw * N], AF.Exp,
                                         bias=mbt1_sb[:, gi:gi + 1])
                    pump(pumps[0] if h0 == 0 else 0)
                pump(pumps[1])
                # keypoint MLP, all heads batched: [100, 800]
                aqv = sT0[0:NK, :].rearrange("p (h t) -> p h t", h=H)[:, :, 0:NK]
                raq = mlp_pool.tile([NK, H * NK], BF16, tag="raq")
                raqv = raq[:, :].rearrange("p (h t) -> p h t", h=H)
                nc.vector.tensor_scalar(raqv, aqv, 0.0, None, ALU.max)
                h1 = mlp_pool.tile([50, H * NK], BF16, tag="h1")
                for c0, cw in ((0, 512), (512, 288)):
                    m1 = pm.tile([128, 512], F32, tag="pm")
                    nc.tensor.matmul(m1[:50, :cw], w1_sb[:, :], raq[:, c0:c0 + cw])
                    nc.vector.tensor_scalar(h1[:, c0:c0 + cw], m1[:50, :cw],
                                            b1_sb[:], 0.0, ALU.add, ALU.max)
                pump(pumps[2])
                lv = mlp_pool.tile([NK, H * NK], BF16, tag="lv")
                for c0, cw in ((0, 512), (512, 288)):
                    m2 = pm.tile([128, 512], F32, tag="pm")
                    nc.tensor.matmul(m2[:NK, :cw], w2_sb[:, :], h1[:, c0:c0 + cw])
                    nc.vector.tensor_scalar(lv[:, c0:c0 + cw], m2[:NK, :cw],
                                            b2_sb[:], 0.0, ALU.add, ALU.max)
                # xin = aq + xian * lv  (xian broadcast across heads); the
                # product lands in raq's tile, which is dead after MLP1
                tmpv = raq[:, :].rearrange("p (h t) -> p h t", h=H)
                lvv = lv[:, :].rearrange("p (h t) -> p h t", h=H)
                xibv = xian_sb[:, b * NK:(b + 1) * NK].unsqueeze(1).broadcast_to((NK, H, NK))
                nc.vector.tensor_mul(tmpv, lvv, xibv)
                nc.vector.tensor_add(aqv, aqv, tmpv)
                # exp over the full main tile
                nc.scalar.activation(probs0[:], sT0[:], AF.Exp)
                state[gi] = (probs0, probs1, outT_p, half, v_sb, b)

            # ---- attention phase B: denominators + AV for batch gi ----
            def emit_B(gi, pumps):
                probs0, probs1, outT_p, half, v_sb, b = state.pop(gi)
                rbc = rbc_pool.tile([128, H * N], F32, tag="rbc")
                for c0, cw in ((0, 512), (512, 512), (1024, 288)):
                    dp = pm.tile([128, 512], F32, tag="pm")
                    nc.tensor.matmul(dp[:, :cw], onesm_sb[:, :],
                                     probs0[:, c0:c0 + cw], start=True, stop=False)
                    nc.tensor.matmul(dp[:, :cw], onesm_sb[:N1, :],
                                     probs1[:, c0:c0 + cw], start=False, stop=True)
                    nc.vector.reciprocal_approx_fast(rbc[:, c0:c0 + cw], dp[:, :cw])
                pump(pumps[0])
                outv = outT_p[:, :].rearrange("p (h t) -> p h t", h=H)
                for h0, gw in HGRP:
                    oT = psc.tile([128, 492], F32, tag="sc")
                    for k in range(gw):
                        h = h0 + k
                        nc.tensor.matmul(oT[:, k * N:k * N + N],
                                         v_sb[:, (b * 2) * D + h * DH:(b * 2) * D + h * DH + DH],
                                         probs0[:, h * N:h * N + N],
                                         start=True, stop=False)
                        nc.tensor.matmul(oT[:, k * N:k * N + N],
                                         v_sb[:N1, (b * 2 + 1) * D + h * DH:(b * 2 + 1) * D + h * DH + DH],
                                         probs1[:, h * N:h * N + N],
                                         start=False, stop=True)
                    nc.vector.tensor_mul(
                        outv[:, h0:h0 + gw, half * N:half * N + N],
                        oT[:, :gw * N].rearrange("p (h t) -> p h t", h=gw),
                        rbc[:, h0 * N:(h0 + gw) * N].rearrange("p (h t) -> p h t", h=gw),
                    )
                    pump(pumps[1] if h0 == 0 else 0)
                if half == 1:
                    p = gi // 2
                    for it in range(3):
                        finals.append(lambda it=it, outT_p=outT_p, p=p:
                                      final_chain(outT_p, p, it))

            # ================= main schedule =================
            # prologue: group 0 inputs, weights, group 0 chains
            g0_chains, _ = start_group(0)
            # first 2 wqk chunks before the rest so chain 0 starts early
            nc.sync.dma_start(wqk_sb[:, :1024], d_wqk.ap()[:, :1024])
            nc.sync.dma_start(wqk_sb[:, 1024:2048], d_wqk.ap()[:, 1024:2048])
            load_consts_early()
            for c in range(2, 16):
                nc.sync.dma_start(wqk_sb[:, c * 1024:(c + 1) * 1024],
                                  d_wqk.ap()[:, c * 1024:(c + 1) * 1024])
            load_consts_late()
            for ch in g0_chains:
                ch()

            last_pair1 = None
            for gi in range(BPC):
                g, b = divmod(gi, GB)
                if b == 0:
                    if g + 1 < NG:
                        chains, tail = start_group(g + 1)
                        work.extend(chains)
                        if tail is not None:
                            last_pair1 = tail
                    elif last_pair1 is not None:
                        work.extend(last_pair1)
                        last_pair1 = None
                # pacing: spread remaining queued chains over the remaining
                # steps of this group window (last group: pair1 chains must
                # land within its first two steps)
                if g < NG - 1:
                    window_left = GB - b
                else:
                    window_left = max(1, 2 - b)
                quota = -(-len(work) // window_left)
                qa = quota // 3
                emit_A(gi, pumps=(qa, qa, quota - 2 * qa))
                if gi > 0:
                    emit_B(gi - 1, pumps=(1, 1))
                pump_finals(2)

            emit_B(BPC - 1, pumps=(0, 0))
            while finals or work:
                pump(99)

    nc.compile()
    return nc


def _prep_core_inputs(xc, maskc, xianc, shared):
    # xT tiles: [BPC,N,D] -> (g, p, dt, b, n) -> [NG, 128, 8*GB*N]
    xt = xc.transpose(0, 2, 1).reshape(NG, GB, 8, 128, N)
    xt = np.ascontiguousarray(xt.transpose(0, 3, 2, 1, 4)).reshape(NG, 128, 8 * XCOLS)
    xt = xt.astype(nbf16)
    # mask bias transposed: [164, BPC]
    mb = np.where(maskc, np.float32(MASK_NEG), np.float32(0.0)).astype(np.float32)
    mbt = np.ascontiguousarray(mb.T)
    # xianT: [BPC,100,100] -> xianT[b][j,i] = xian[b][i,j] -> (g, j, b, i)
    xiant = xianc.transpose(0, 2, 1).reshape(NG, GB, NK, NK)
    xiant = np.ascontiguousarray(xiant.transpose(0, 2, 1, 3)).reshape(NG, NK, GB * NK)
    xiant = xiant.astype(nbf16)
    m = {
        "xt": xt,
        "mbt0": np.ascontiguousarray(mbt[:128]),
        "mbt1": np.ascontiguousarray(mbt[128:]),
        "xiant": xiant,
    }
    m.update(shared)
    return m


def kernel(x, mask, xian, Wqkv, W1, b1, W2, b2, Wout, bout):
    global LAST_EXEC_NS
    x = np.asarray(x, dtype=np.float32)
    mask = np.asarray(mask)
    xian = np.asarray(xian, dtype=np.float32)
    Wqkv = np.asarray(Wqkv, dtype=np.float32)
    W1 = np.asarray(W1, dtype=np.float32)
    b1 = np.asarray(b1, dtype=np.float32)
    W2 = np.asarray(W2, dtype=np.float32)
    b2 = np.asarray(b2, dtype=np.float32)
    Wout = np.asarray(Wout, dtype=np.float32)
    bout = np.asarray(bout, dtype=np.float32)

    if "nc" not in _CACHE:
        _CACHE["nc"] = _build_nc()
    nc = _CACHE["nc"]

    # ---- shared weight prep (scale folded into Wq) ----
    scale = np.float32(D ** -0.5)
    wqkv_s = Wqkv.copy()
    wqkv_s[:, :D] *= scale
    # wqk ct-major: [dt,p, ct,c] -> [p, ct, dt, c]
    wqk = wqkv_s[:, :2 * D].reshape(8, 128, 16, 128)
    wqk_h = np.ascontiguousarray(wqk.transpose(1, 2, 0, 3)).reshape(128, 16 * 1024).astype(nbf16)
    # wv dt-major: [dt, p, c] -> [p, dt, c]
    wv = wqkv_s[:, 2 * D:].reshape(8, 128, 1024)
    wv_h = np.ascontiguousarray(wv.transpose(1, 0, 2)).reshape(128, 8 * 1024).astype(nbf16)
    wout_h = np.ascontiguousarray(
        Wout.reshape(8, 128, D).transpose(1, 0, 2)).reshape(128, 8 * D).astype(nbf16)
    shared = {
        "wqk": wqk_h,
        "wv": wv_h,
        "wout": wout_h,
        "w1": W1.astype(nbf16),
        "w2": W2.astype(nbf16),
        "b1c": np.ascontiguousarray(b1.reshape(50, 1)),
        "b2c": np.ascontiguousarray(b2.reshape(NK, 1)),
        "boutb": np.ascontiguousarray(np.broadcast_to(bout, (128, D))).astype(np.float32),
    }

    in_maps = []
    for c in range(NCORES):
        sl = slice(c * BPC, (c + 1) * BPC)
        in_maps.append(_prep_core_inputs(x[sl], mask[sl], xian[sl], shared))

    trace = bool(int(os.environ.get("KERNEL_TRACE", "0")))
    if trace:
        trace = _install_profile_hook()
    res = bass_utils.run_bass_kernel_spmd(
        nc, in_maps, core_ids=list(range(NCORES)), trace=trace)
    LAST_EXEC_NS = res.exec_time_ns

    out = np.empty((B, N, D), dtype=np.float32)
    for c in range(NCORES):
        out[c * BPC:(c + 1) * BPC] = res.results[c]["y"].reshape(BPC, N, D).astype(np.float32)
    return out
